# revision 8
# baseline (speedup 1.0000x reference)
"""Adaptive embedding lookup (nn.AdaptiveEmbedding) on 8 TRN2 NeuronCores.

Strategy (data-parallel over tokens, tables replicated, no collectives):

Host:
  - Bucket the 16384 tokens by embedding cluster; deal each bucket's
    tokens round-robin to the 8 cores; pad each per-core bucket to a
    multiple of 128 (one PE tile = one output "group" of 128 tokens).
  - Tables bf16.  c0/c1 are gathered with gpsimd indirect DMA (int32 row
    indices, row-per-partition).  c2/c3 go through the dma_gather ucode
    (transpose=True): c2 is split into 5 sub-ranges of 32000 rows (int16
    index limit) with rows zero-padded to 128 elems (256B); c3 is packed
    8-rows-per-256B-super-row with a mask selecting the sub-row.
  - Projections pre-transposed, pre-scaled by sqrt(d_proj), bf16,
    chunk-major; identity/pt2/pt3s/c3-masks packed into one "cst" param.

Device (SPMD, identical graph on all 8 cores, one TileContext):
  - The dma_gather ucode library load blocks c2/c3 gathers until ~21us
    in.  That window is filled with REAL work: c0/c1 rows are fetched by
    indirect_dma_start (base SWDGE path, no ucode library), PE-transposed
    ([token,dim] -> [dim,token] lhsT via the identity trick), and fully
    projected — so the PE is busy with c0/c1 matmuls while the library
    loads, and c2/c3 matmuls follow seamlessly once their gathers land.
  - Per 128-token group: matmuls accumulate into a 2-bank PSUM tile, then
    ONE [128,1024] f32->bf16 copy into a per-unit staging tile
    (vector/scalar alternating).
  - Out DMAs are batched per 2-3 groups ([128, k, 1024] SBUF ->
    [k*128, 1024] HBM rows), sync-issued — per-issue cost (~0.7us)
    dominated per-group DMAs.

Host: inverse-permute the 8 per-core outputs into [8, 2048, 1024] f32.
"""

import numpy as np
import ml_dtypes

import concourse.bacc as bacc
import concourse.bass as bass
import concourse.mybir as mybir
from concourse.bass_utils import run_bass_kernel_spmd
from concourse.tile import TileContext

N_TOKEN = 267735
D_PROJ = 1024
CUTOFF_ENDS = [0, 20000, 40000, 200000, 267735]
D_EMBS = [1024, 256, 64, 16]
EMB_SCALE = float(D_PROJ) ** 0.5
N_CORES = 8
P = 128
NFREE = 512          # psum free-dim per matmul
C2_SUB = 32000       # cluster-2 subtable rows (int16 range)
C2_NSUB = 5
C3_PACK = 8          # cluster-3 rows packed per super-row
C3_SROWS = -(-(CUTOFF_ENDS[4] - CUTOFF_ENDS[3]) // C3_PACK)  # 8467

BF16 = ml_dtypes.bfloat16

# Test-harness knobs (the grader never touches these).
TRACE = False
TRACE_CORES = None
LAST = {}

_GRAPH_CACHE = {}

# unit = gather bucket: 0, 1, (2, r) for sub-range r, 3.
UNIT_KEYS = [0, 1] + [(2, r) for r in range(C2_NSUB)] + [3]
UNITS_16 = [(2, r) for r in range(C2_NSUB)] + [3]   # dma_gather (idx16) units

N_WARMUP = 24        # PE warmup matmuls (HAM ramp) before the first work


def _build_graph(Ks):
    """Ks: dict unit_key -> group count (0 allowed). Same on all cores."""
    key = tuple(Ks[u] for u in UNIT_KEYS)
    if key in _GRAPH_CACHE:
        return _GRAPH_CACHE[key]

    K0, K1, K3 = Ks[0], Ks[1], Ks[3]
    NI32 = K0 + K1                     # idx32 columns (1 per c0/c1 group)
    NI16 = 8 * sum(Ks[u] for u in UNITS_16)
    G = sum(Ks.values())               # total output groups
    # cst param: [identity | c3 masks | pt2 | pt3s]
    NCST = P + max(K3, 1) * P + D_PROJ + D_PROJ

    nc = bacc.Bacc("TRN2", debug=False, num_swdge_queues=4)
    idx32_ext = nc.declare_dram_parameter("idx32", [P, max(NI32, 1)], mybir.dt.int32, False)
    idx16_ext = nc.declare_dram_parameter("idx16", [P, max(NI16, 16)], mybir.dt.int16, False)
    emb0_ext = nc.declare_dram_parameter("emb0b", [20000, 1024], mybir.dt.bfloat16, False)
    emb1_ext = nc.declare_dram_parameter("emb1b", [20000, 256], mybir.dt.bfloat16, False)
    emb2_ext = nc.declare_dram_parameter("emb2p", [C2_SUB * C2_NSUB, 128], mybir.dt.bfloat16, False)
    emb3_ext = nc.declare_dram_parameter("emb3p", [C3_SROWS, 128], mybir.dt.bfloat16, False)
    pt0_ext = nc.declare_dram_parameter("pt0", [128, 8, D_PROJ], mybir.dt.bfloat16, False)
    pt1_ext = nc.declare_dram_parameter("pt1", [128, 2, D_PROJ], mybir.dt.bfloat16, False)
    cst_ext = nc.declare_dram_parameter("cst", [P, NCST], mybir.dt.bfloat16, False)
    out_ext = nc.declare_dram_parameter("out", [G * P, D_PROJ], mybir.dt.bfloat16, True)

    import bass_rust as _br

    with TileContext(nc) as tc:
        with tc.tile_pool(name="const", bufs=1) as constp, \
             tc.tile_pool(name="ps_o", bufs=3, space="PSUM") as psump, \
             tc.tile_pool(name="ps_t", bufs=2, space="PSUM") as psumtr:
            # index DMAs first: the gathers wait on these
            idx32_sb = constp.tile([P, max(NI32, 1)], mybir.dt.int32, tag="idx32")
            nc.sync.dma_start(out=idx32_sb[:], in_=idx32_ext[:])
            idx16_sb = constp.tile([P, max(NI16, 16)], mybir.dt.int16, tag="idx16")
            nc.sync.dma_start(out=idx16_sb[:], in_=idx16_ext[:])
            cst_sb = constp.tile([P, NCST], mybir.dt.bfloat16, tag="cst")
            nc.scalar.dma_start(out=cst_sb[:], in_=cst_ext[:])
            ident = cst_sb[:, 0:P]
            mask_v = cst_sb[:, P:P + max(K3, 1) * P]
            pt2_sb = cst_sb[:, P + max(K3, 1) * P:P + max(K3, 1) * P + D_PROJ]
            pt3_sb = cst_sb[:, P + max(K3, 1) * P + D_PROJ:]

            # c0/c1 indirect gathers (base SWDGE path — no ucode library):
            # row-per-partition [token, dim] tiles
            eg0 = constp.tile([P, max(K0, 1), 1024], mybir.dt.bfloat16, tag="eg0")
            eg1 = constp.tile([P, max(K1, 1), 256], mybir.dt.bfloat16, tag="eg1")
            for g in range(K0):
                nc.gpsimd.indirect_dma_start(
                    out=eg0[:, g, :], out_offset=None,
                    in_=emb0_ext[:],
                    in_offset=bass.IndirectOffsetOnAxis(
                        ap=idx32_sb[:, g:g + 1], axis=0),
                )
            for g in range(K1):
                nc.gpsimd.indirect_dma_start(
                    out=eg1[:, g, :], out_offset=None,
                    in_=emb1_ext[:],
                    in_offset=bass.IndirectOffsetOnAxis(
                        ap=idx32_sb[:, K0 + g:K0 + g + 1], axis=0),
                )

            # c2/c3 transpose dma_gathers (ucode; start once the library is
            # loaded ~21us in), per group, spread over the 4 SWDGE queues
            e2s = [
                constp.tile([P, max(Ks[(2, r)], 1), 1, P], mybir.dt.bfloat16,
                            tag=f"e2_{r}", name=f"e2_{r}")
                for r in range(C2_NSUB)
            ]
            e3 = constp.tile([P, max(K3, 1), 1, P], mybir.dt.bfloat16, tag="e3")
            em = constp.tile([P, max(K3, 1) * P], mybir.dt.bfloat16, tag="em")

            unit_col = {}
            col = 0
            for u in UNITS_16:
                unit_col[u] = col
                col += 8 * Ks[u]

            qn = 0
            for r in range(C2_NSUB):
                for j in range(Ks[(2, r)]):
                    c0_ = unit_col[(2, r)] + 8 * j
                    nc.gpsimd.dma_gather(
                        e2s[r][:, j, :, :],
                        emb2_ext[r * C2_SUB:(r + 1) * C2_SUB, :],
                        idx16_sb[:, c0_:c0_ + 8], P, P, 128,
                        transpose=True, queue_num=qn % 4,
                    )
                    qn += 1
            for j in range(K3):
                c0_ = unit_col[3] + 8 * j
                nc.gpsimd.dma_gather(
                    e3[:, j, :, :], emb3_ext[:],
                    idx16_sb[:, c0_:c0_ + 8], P, P, 128,
                    transpose=True, queue_num=qn % 4,
                )
                qn += 1

            # projections; pt0 split in two so chunk-0..3 matmuls can start
            # before chunks 4..7 land
            pt0_sb = constp.tile([128, 8, D_PROJ], mybir.dt.bfloat16, tag="pt0")
            nc.sync.dma_start(out=pt0_sb[:, 0:4, :], in_=pt0_ext[:, 0:4, :])
            nc.sync.dma_start(out=pt0_sb[:, 4:8, :], in_=pt0_ext[:, 4:8, :])
            pt1_sb = constp.tile([128, 2, D_PROJ], mybir.dt.bfloat16, tag="pt1")
            nc.sync.dma_start(out=pt1_sb[:], in_=pt1_ext[:])

            # PE warmup on a memset tile (HAM clock ramp)
            wtile = constp.tile([P, P], mybir.dt.bfloat16, tag="wm")
            nc.vector.memset(wtile[:], 1.0)
            last_pe_inst = [None]
            wps = psump.tile([P, 2 * NFREE], mybir.dt.float32, tag="ps")
            for _ in range(N_WARMUP):
                mm = nc.tensor.matmul(
                    out=wps[:, 0:P], lhsT=wtile[:], rhs=wtile[:],
                    start=True, stop=True,
                )
                last_pe_inst[0] = mm.ins

            def pe_chain(inst):
                if last_pe_inst[0] is not None:
                    _br.add_dep_helper(
                        inst, last_pe_inst[0], sync=False,
                        reason="keep PE stream in data-landing order",
                    )
                last_pe_inst[0] = inst

            # PE transposes of the c0/c1 indirect-gathered tiles:
            # [token, dim-chunk] -> [dim, token] lhsT staging
            e0T = constp.tile([P, max(K0, 1), 8, P], mybir.dt.bfloat16, tag="e0T")
            e1T = constp.tile([P, max(K1, 1), 2, P], mybir.dt.bfloat16, tag="e1T")
            tr_ct = [0]

            def emit_tr(src, dst):
                tp = psumtr.tile([P, P], mybir.dt.bfloat16, tag="tr")
                ti = nc.tensor.transpose(out=tp[:], in_=src, identity=ident)
                pe_chain(ti.ins)
                if tr_ct[0] % 2 == 0:
                    nc.vector.tensor_copy(out=dst, in_=tp[:])
                else:
                    nc.scalar.copy(out=dst, in_=tp[:])
                tr_ct[0] += 1

            for g in range(K0):
                for kc in range(8):
                    emit_tr(eg0[:, g, kc * P:(kc + 1) * P], e0T[:, g, kc, :])
            for g in range(K1):
                for kc in range(2):
                    emit_tr(eg1[:, g, kc * P:(kc + 1) * P], e1T[:, g, kc, :])

            # per-unit staging tiles for the batched out DMAs
            stage = {}
            for u in UNIT_KEYS:
                stage[u] = constp.tile(
                    [P, max(Ks[u], 1), D_PROJ], mybir.dt.bfloat16,
                    tag=f"st{u}", name=f"st_{u}",
                )

            gbase_dev = {}
            acc_g = 0
            for u in UNIT_KEYS:
                gbase_dev[u] = acc_g
                acc_g += Ks[u]

            cp_ct = [0]   # copy-engine round robin
            out_ct = [0]  # out-DMA engine round robin

            def emit_group(ch, lhsT_of, rhs_of, st, j):
                ps = psump.tile([P, 2 * NFREE], mybir.dt.float32, tag="ps")
                for kc in range(ch):
                    lt = lhsT_of(kc)
                    for oc in range(2):
                        mm = nc.tensor.matmul(
                            out=ps[:, oc * NFREE:(oc + 1) * NFREE],
                            lhsT=lt,
                            rhs=rhs_of(kc, oc),
                            start=(kc == 0),
                            stop=(kc == ch - 1),
                        )
                        pe_chain(mm.ins)
                dst = st[:, j, :]
                if cp_ct[0] % 2 == 0:
                    nc.vector.tensor_copy(out=dst, in_=ps[:])
                else:
                    nc.scalar.copy(out=dst, in_=ps[:])
                cp_ct[0] += 1

            def emit_out(u, j0, k):
                """DMA groups [j0, j0+k) of unit u to HBM."""
                base = (gbase_dev[u] + j0) * P
                dst = out_ext[base:base + k * P, :].rearrange(
                    "(q p) d -> p q d", p=P
                )
                eng = nc.sync if out_ct[0] % 2 == 0 else nc.scalar
                out_ct[0] += 1
                eng.dma_start(out=dst, in_=stage[u][:, j0:j0 + k, :])

            def emit_unit_outs(u):
                K = Ks[u]
                j0 = 0
                while j0 < K:
                    k = min(2, K - j0)
                    # avoid a trailing singleton unless the unit is size 1
                    if K - j0 == 3:
                        k = 3
                    emit_out(u, j0, k)
                    j0 += k

            # ---- c0 (8 contraction chunks) ----
            for j in range(K0):
                emit_group(
                    8,
                    lambda kc, _j=j: e0T[:, _j, kc, :],
                    lambda kc, oc: pt0_sb[:, kc, oc * NFREE:(oc + 1) * NFREE],
                    stage[0], j,
                )
            emit_unit_outs(0)
            # ---- c1 (2 chunks) ----
            for j in range(K1):
                emit_group(
                    2,
                    lambda kc, _j=j: e1T[:, _j, kc, :],
                    lambda kc, oc: pt1_sb[:, kc, oc * NFREE:(oc + 1) * NFREE],
                    stage[1], j,
                )
            emit_unit_outs(1)
            # ---- c2 (contraction 64; top 64 partitions of the padded rows
            # are unread) ----
            for r in range(C2_NSUB):
                for j in range(Ks[(2, r)]):
                    emit_group(
                        1,
                        lambda kc, _r=r, _j=j: e2s[_r][:64, _j, 0, :],
                        lambda kc, oc: pt2_sb[:64, oc * NFREE:(oc + 1) * NFREE],
                        stage[(2, r)], j,
                    )
                emit_unit_outs((2, r))
            # ---- c3: per-group mask-select of the 16-elem sub-row inside
            # the 128-elem packed super-row, then matmul vs the 8x-tiled
            # projection ----
            for j in range(K3):
                nc.vector.tensor_tensor(
                    out=em[:, j * P:(j + 1) * P],
                    in0=e3[:, j, 0, :],
                    in1=mask_v[:, j * P:(j + 1) * P],
                    op=mybir.AluOpType.mult,
                )
                emit_group(
                    1,
                    lambda kc, _j=j: em[:, _j * P:(_j + 1) * P],
                    lambda kc, oc: pt3_sb[:, oc * NFREE:(oc + 1) * NFREE],
                    stage[3], j,
                )
            emit_unit_outs(3)

    nc.compile()
    _GRAPH_CACHE[key] = nc
    return nc


def _wrap_idx16(vals, n_slots):
    """int16 values (len <= n_slots, padded with 0) -> [128, n_slots/16] wrapped."""
    full = np.zeros(n_slots, dtype=np.int16)
    full[:len(vals)] = vals
    w = np.zeros((16, n_slots // 16), dtype=np.int16)
    m = np.arange(n_slots)
    w[m % 16, m // 16] = full
    return np.tile(w, (8, 1))


def kernel(inp, emb0, emb1, emb2, emb3, proj0, proj1, proj2, proj3):
    inp = np.asarray(inp)
    embs = [np.asarray(e) for e in (emb0, emb1, emb2, emb3)]
    projs = [np.asarray(p) for p in (proj0, proj1, proj2, proj3)]
    B, S = inp.shape
    flat = inp.reshape(-1).astype(np.int64)
    T = flat.shape[0]

    # ---- host-side bucketing -------------------------------------------
    flat = np.clip(flat, 0, N_TOKEN - 1)
    cluster = np.clip(
        np.searchsorted(np.asarray(CUTOFF_ENDS[1:]), flat, side="right"), 0, 3
    )
    local = flat - np.asarray(CUTOFF_ENDS)[cluster]

    unit_pos = {}
    for u in UNIT_KEYS:
        if u == 0 or u == 1 or u == 3:
            unit_pos[u] = np.nonzero(cluster == u)[0]
        else:
            r = u[1]
            unit_pos[u] = np.nonzero((cluster == 2) & (local // C2_SUB == r))[0]

    core_lists = {u: [unit_pos[u][k::N_CORES] for k in range(N_CORES)]
                  for u in UNIT_KEYS}
    Ks = {
        u: int(-(-max(len(core_lists[u][k]) for k in range(N_CORES)) // P))
        for u in UNIT_KEYS
    }
    G = sum(Ks.values())
    K0, K1, K3 = Ks[0], Ks[1], Ks[3]

    NI32 = K0 + K1
    NI16 = 8 * sum(Ks[u] for u in UNITS_16)
    gbase = {}
    acc = 0
    for u in UNIT_KEYS:
        gbase[u] = acc
        acc += Ks[u]

    NCST = P + max(K3, 1) * P + D_PROJ + D_PROJ
    blkid = np.arange(128) // 16  # sub-row block of each super-row element

    # identity / pt2 / pt3s are core-independent
    pt2pad = np.zeros((P, D_PROJ), dtype=np.float32)
    pt2pad[:64] = projs[2].T.astype(np.float32) * EMB_SCALE
    pt3s = np.tile(projs[3].T.astype(np.float32) * EMB_SCALE, (C3_PACK, 1))

    idx32_maps, idx16_maps, cst_maps, row_maps = [], [], [], []
    for k in range(N_CORES):
        cols16 = []
        idx32 = np.zeros((P, max(NI32, 1)), dtype=np.int32)
        row_map = np.full(G * P, -1, dtype=np.int64)
        cst = np.zeros((P, NCST), dtype=np.float32)
        cst[:, 0:P] = np.eye(P, dtype=np.float32)
        cst[:, P + max(K3, 1) * P:P + max(K3, 1) * P + D_PROJ] = pt2pad
        cst[:, P + max(K3, 1) * P + D_PROJ:] = pt3s
        for u in UNIT_KEYS:
            n = Ks[u]
            if n == 0:
                continue
            lst = core_lists[u][k]
            m = np.arange(len(lst))
            row_map[(gbase[u] + m // P) * P + (m % P)] = lst
            if u == 0 or u == 1:
                base = 0 if u == 0 else K0
                full = np.zeros(n * P, dtype=np.int32)
                full[:len(lst)] = local[lst]
                idx32[:, base:base + n] = full.reshape(n, P).T
            else:
                lv = local[lst]
                if u == 3:
                    cols16.append(_wrap_idx16(
                        (lv // C3_PACK).astype(np.int16), n * P))
                    s_arr = lv % C3_PACK
                    mask = np.zeros((P, K3 * P), dtype=np.float32)
                    mask[:, m] = (blkid[:, None] == s_arr[None, :])
                    cst[:, P:P + K3 * P] = mask
                else:
                    cols16.append(_wrap_idx16(
                        (lv - u[1] * C2_SUB).astype(np.int16), n * P))
        idx16 = (np.concatenate(cols16, axis=1) if cols16
                 else np.zeros((P, 16), np.int16))
        if idx16.shape[1] < max(NI16, 16):
            pad = np.zeros((P, max(NI16, 16) - idx16.shape[1]), np.int16)
            idx16 = np.concatenate([idx16, pad], axis=1)
        idx32_maps.append(np.ascontiguousarray(idx32))
        idx16_maps.append(np.ascontiguousarray(idx16))
        cst_maps.append(cst.astype(BF16))
        row_maps.append(row_map)

    # ---- table/projection prep -----------------------------------------
    emb0b = np.ascontiguousarray(embs[0].astype(BF16))
    emb1b = np.ascontiguousarray(embs[1].astype(BF16))
    emb2p = np.zeros((C2_SUB * C2_NSUB, 128), dtype=BF16)
    emb2p[:160000, :64] = embs[2].astype(BF16)
    e3flat = embs[3].astype(np.float32)
    pad3 = C3_SROWS * C3_PACK - e3flat.shape[0]
    e3flat = np.concatenate([e3flat, np.zeros((pad3, 16), np.float32)], axis=0)
    emb3p = np.ascontiguousarray(e3flat.reshape(C3_SROWS, 128).astype(BF16))

    pts = {}
    for c, name, pc, ch in ((0, "pt0", 128, 8), (1, "pt1", 128, 2)):
        ptc = (projs[c].T.astype(np.float32) * EMB_SCALE).astype(BF16)
        pts[name] = np.ascontiguousarray(
            ptc.reshape(ch, pc, D_PROJ).transpose(1, 0, 2)
        )

    in_maps = []
    for k in range(N_CORES):
        m = {
            "idx32": idx32_maps[k], "idx16": idx16_maps[k], "cst": cst_maps[k],
            "emb0b": emb0b, "emb1b": emb1b, "emb2p": emb2p, "emb3p": emb3p,
        }
        m.update(pts)
        in_maps.append(m)

    # ---- device --------------------------------------------------------
    nc = _build_graph(Ks)
    res = run_bass_kernel_spmd(
        nc,
        in_maps,
        core_ids=list(range(N_CORES)),
        trace=TRACE,
        trace_cores=TRACE_CORES,
    )
    LAST["res"] = res
    LAST["Ks"] = Ks

    # ---- host-side unshard ---------------------------------------------
    out_full = np.zeros((T, D_PROJ), dtype=np.float32)
    for k in range(N_CORES):
        o = np.asarray(res.results[k]["out"])
        rm = row_maps[k]
        valid = rm >= 0
        out_full[rm[valid]] = o[valid].astype(np.float32)
    return out_full.reshape(B, S, D_PROJ)


# revision 9
# speedup vs baseline: 1.3577x; 1.3577x over previous
"""Adaptive embedding lookup (nn.AdaptiveEmbedding) on 8 TRN2 NeuronCores.

Strategy (data-parallel over tokens, tables replicated, no collectives):

Host:
  - Bucket the 16384 tokens by embedding cluster; deal each bucket's
    tokens round-robin to the 8 cores; pad each per-core bucket to a
    multiple of 128 (one PE tile = one output "group" of 128 tokens).
  - Tables bf16.  c0/c1 are gathered with gpsimd indirect DMA (int32 row
    indices, row-per-partition, base SWDGE path — no ucode library).
    c2/c3 go through the dma_gather ucode (transpose=True): c2 split into
    5 sub-ranges of 32000 rows (int16 index limit) with rows zero-padded
    to 128 elems (256B); c3 packed 8-rows-per-256B-super-row with a mask
    selecting the sub-row.
  - Projections pre-transposed, pre-scaled by sqrt(d_proj), bf16,
    chunk-major; identity/pt2/pt3s/c3-masks packed into one "cst" param.

Device (SPMD, identical graph on all 8 cores, one TileContext):
  - The dma_gather ucode library load (~14us) is kicked explicitly as the
    FIRST gpsimd op; while it loads, the c0/c1 indirect gathers, their PE
    transposes ([token,dim] -> [dim,token] via the identity trick, 8
    transposes banked into one 2KB PSUM bank -> ONE bf16 copy out), and
    the full c0/c1 projection matmuls run — so the window is real work.
    c2/c3 transpose-gathers follow once the library lands.
  - Per 128-token group: matmuls accumulate into a 2-bank PSUM tile, then
    ONE [128,1024] f32->bf16 copy into a per-unit staging tile
    (vector/scalar alternating).  Vector and scalar streams are pinned in
    emission order (no-sync deps) — the scheduler otherwise hoists ops
    whose inputs land last, head-of-line-blocking the engine.
  - Out DMAs batched per 2 groups ([128, k, 1024] SBUF -> [k*128, 1024]
    HBM rows), sync/scalar alternating.

Host: inverse-permute the 8 per-core outputs into [8, 2048, 1024] f32.
"""

import numpy as np
import ml_dtypes

import concourse.bacc as bacc
import concourse.bass as bass
import concourse.mybir as mybir
from concourse import library_config
from concourse.bass_utils import run_bass_kernel_spmd
from concourse.tile import TileContext

N_TOKEN = 267735
D_PROJ = 1024
CUTOFF_ENDS = [0, 20000, 40000, 200000, 267735]
D_EMBS = [1024, 256, 64, 16]
EMB_SCALE = float(D_PROJ) ** 0.5
N_CORES = 8
P = 128
NFREE = 512          # psum free-dim per matmul
C2_SUB = 32000       # cluster-2 subtable rows (int16 range)
C2_NSUB = 5
C3_PACK = 8          # cluster-3 rows packed per super-row
C3_SROWS = -(-(CUTOFF_ENDS[4] - CUTOFF_ENDS[3]) // C3_PACK)  # 8467

BF16 = ml_dtypes.bfloat16

# Test-harness knobs (the grader never touches these).
TRACE = False
TRACE_CORES = None
LAST = {}

_GRAPH_CACHE = {}

# unit = gather bucket: 0, 1, (2, r) for sub-range r, 3.
UNIT_KEYS = [0, 1] + [(2, r) for r in range(C2_NSUB)] + [3]
UNITS_16 = [(2, r) for r in range(C2_NSUB)] + [3]   # dma_gather (idx16) units

N_WARMUP = 28        # PE warmup matmuls (HAM ramp) before the first work


def _build_graph(Ks):
    """Ks: dict unit_key -> group count (0 allowed). Same on all cores."""
    key = tuple(Ks[u] for u in UNIT_KEYS)
    if key in _GRAPH_CACHE:
        return _GRAPH_CACHE[key]

    K0, K1, K3 = Ks[0], Ks[1], Ks[3]
    NI32 = K0 + K1                     # idx32 columns (1 per c0/c1 group)
    NI16 = 8 * sum(Ks[u] for u in UNITS_16)
    G = sum(Ks.values())               # total output groups
    # cst param: [identity | c3 masks | pt2 | pt3s]
    NCST = P + max(K3, 1) * P + D_PROJ + D_PROJ

    nc = bacc.Bacc("TRN2", debug=False, num_swdge_queues=4)
    idx32_ext = nc.declare_dram_parameter("idx32", [P, max(NI32, 1)], mybir.dt.int32, False)
    idx16_ext = nc.declare_dram_parameter("idx16", [P, max(NI16, 16)], mybir.dt.int16, False)
    emb0_ext = nc.declare_dram_parameter("emb0b", [20000, 1024], mybir.dt.bfloat16, False)
    emb1_ext = nc.declare_dram_parameter("emb1b", [20000, 256], mybir.dt.bfloat16, False)
    emb2_ext = nc.declare_dram_parameter("emb2p", [C2_SUB * C2_NSUB, 128], mybir.dt.bfloat16, False)
    emb3_ext = nc.declare_dram_parameter("emb3p", [C3_SROWS, 128], mybir.dt.bfloat16, False)
    pt0_ext = nc.declare_dram_parameter("pt0", [128, 8, D_PROJ], mybir.dt.bfloat16, False)
    pt1_ext = nc.declare_dram_parameter("pt1", [128, 2, D_PROJ], mybir.dt.bfloat16, False)
    cst_ext = nc.declare_dram_parameter("cst", [P, NCST], mybir.dt.bfloat16, False)
    out_ext = nc.declare_dram_parameter("out", [G * P, D_PROJ], mybir.dt.bfloat16, True)

    import bass_rust as _br

    with TileContext(nc) as tc:
        with tc.tile_pool(name="const", bufs=1) as constp, \
             tc.tile_pool(name="ps_o", bufs=3, space="PSUM") as psump, \
             tc.tile_pool(name="ps_t", bufs=2, space="PSUM") as psumtr:
            # kick the dma_gather ucode library load before anything else
            # on gpsimd — it takes ~14us and gates the c2/c3 gathers
            nc.gpsimd.load_library(library_config.mlp)

            # index DMAs first: the gathers wait on these
            idx32_sb = constp.tile([P, max(NI32, 1)], mybir.dt.int32, tag="idx32")
            nc.sync.dma_start(out=idx32_sb[:], in_=idx32_ext[:])
            idx16_sb = constp.tile([P, max(NI16, 16)], mybir.dt.int16, tag="idx16")
            nc.sync.dma_start(out=idx16_sb[:], in_=idx16_ext[:])
            cst_sb = constp.tile([P, NCST], mybir.dt.bfloat16, tag="cst")
            nc.scalar.dma_start(out=cst_sb[:], in_=cst_ext[:])
            ident = cst_sb[:, 0:P]
            mask_v = cst_sb[:, P:P + max(K3, 1) * P]
            pt2_sb = cst_sb[:, P + max(K3, 1) * P:P + max(K3, 1) * P + D_PROJ]
            pt3_sb = cst_sb[:, P + max(K3, 1) * P + D_PROJ:]

            # per-engine no-sync order chains: pin vector/scalar/tensor
            # streams to emission order so the scheduler cannot hoist a
            # late-data op to the head of an engine queue
            last_inst = {"pe": None, "vec": None, "sca": None}

            def chain(tag, inst):
                if last_inst[tag] is not None:
                    _br.add_dep_helper(
                        inst, last_inst[tag], sync=False,
                        reason=f"pin {tag} stream order",
                    )
                last_inst[tag] = inst

            def vec_op(fn, **kw):
                chain("vec", fn(**kw).ins)

            def sca_op(fn, **kw):
                chain("sca", fn(**kw).ins)

            # c0/c1 indirect gathers (base SWDGE path): row-per-partition
            # [token, dim] tiles
            eg0 = constp.tile([P, max(K0, 1), 1024], mybir.dt.bfloat16, tag="eg0")
            eg1 = constp.tile([P, max(K1, 1), 256], mybir.dt.bfloat16, tag="eg1")
            for g in range(K0):
                nc.gpsimd.indirect_dma_start(
                    out=eg0[:, g, :], out_offset=None,
                    in_=emb0_ext[:],
                    in_offset=bass.IndirectOffsetOnAxis(
                        ap=idx32_sb[:, g:g + 1], axis=0),
                )
            for g in range(K1):
                nc.gpsimd.indirect_dma_start(
                    out=eg1[:, g, :], out_offset=None,
                    in_=emb1_ext[:],
                    in_offset=bass.IndirectOffsetOnAxis(
                        ap=idx32_sb[:, K0 + g:K0 + g + 1], axis=0),
                )

            # c2/c3 transpose dma_gathers (ucode), per group, spread over
            # the 4 SWDGE queues
            e2s = [
                constp.tile([P, max(Ks[(2, r)], 1), 1, P], mybir.dt.bfloat16,
                            tag=f"e2_{r}", name=f"e2_{r}")
                for r in range(C2_NSUB)
            ]
            e3 = constp.tile([P, max(K3, 1), 1, P], mybir.dt.bfloat16, tag="e3")
            em = constp.tile([P, max(K3, 1) * P], mybir.dt.bfloat16, tag="em")

            unit_col = {}
            col = 0
            for u in UNITS_16:
                unit_col[u] = col
                col += 8 * Ks[u]

            qn = 0
            for r in range(C2_NSUB):
                for j in range(Ks[(2, r)]):
                    c0_ = unit_col[(2, r)] + 8 * j
                    nc.gpsimd.dma_gather(
                        e2s[r][:, j, :, :],
                        emb2_ext[r * C2_SUB:(r + 1) * C2_SUB, :],
                        idx16_sb[:, c0_:c0_ + 8], P, P, 128,
                        transpose=True, queue_num=qn % 4,
                    )
                    qn += 1
            for j in range(K3):
                c0_ = unit_col[3] + 8 * j
                nc.gpsimd.dma_gather(
                    e3[:, j, :, :], emb3_ext[:],
                    idx16_sb[:, c0_:c0_ + 8], P, P, 128,
                    transpose=True, queue_num=qn % 4,
                )
                qn += 1

            # projections; pt0 split in two so chunk-0..3 matmuls can start
            # before chunks 4..7 land
            pt0_sb = constp.tile([128, 8, D_PROJ], mybir.dt.bfloat16, tag="pt0")
            nc.sync.dma_start(out=pt0_sb[:, 0:4, :], in_=pt0_ext[:, 0:4, :])
            nc.sync.dma_start(out=pt0_sb[:, 4:8, :], in_=pt0_ext[:, 4:8, :])
            pt1_sb = constp.tile([128, 2, D_PROJ], mybir.dt.bfloat16, tag="pt1")
            nc.sync.dma_start(out=pt1_sb[:], in_=pt1_ext[:])

            # PE warmup on a memset tile (HAM clock ramp)
            wtile = constp.tile([P, P], mybir.dt.bfloat16, tag="wm")
            vec_op(nc.vector.memset, ap=wtile[:], constant=1.0)
            wps = psump.tile([P, 2 * NFREE], mybir.dt.float32, tag="ps")
            for _ in range(N_WARMUP):
                mm = nc.tensor.matmul(
                    out=wps[:, 0:P], lhsT=wtile[:], rhs=wtile[:],
                    start=True, stop=True,
                )
                chain("pe", mm.ins)

            # PE transposes of the c0/c1 indirect-gathered tiles, banked 8
            # per 2KB PSUM bank, one bf16 copy per bank into lhsT staging
            e0T = constp.tile([P, max(K0, 1), 8, P], mybir.dt.bfloat16, tag="e0T")
            e1T = constp.tile([P, max(K1, 1), 2, P], mybir.dt.bfloat16, tag="e1T")
            cp_flip = [0]

            def bank_copy(src_ap, dst_ap):
                if cp_flip[0] % 2 == 0:
                    vec_op(nc.vector.tensor_copy, out=dst_ap, in_=src_ap)
                else:
                    sca_op(nc.scalar.copy, out=dst_ap, in_=src_ap)
                cp_flip[0] += 1

            for g in range(K0):
                tp = psumtr.tile([P, 8, P], mybir.dt.bfloat16, tag="tr")
                for kc in range(8):
                    ti = nc.tensor.transpose(
                        out=tp[:, kc, :],
                        in_=eg0[:, g, kc * P:(kc + 1) * P],
                        identity=ident,
                    )
                    chain("pe", ti.ins)
                bank_copy(tp[:], e0T[:, g, :, :])
            for g0 in range(0, K1, 4):
                gn = min(4, K1 - g0)
                tp = psumtr.tile([P, 8, P], mybir.dt.bfloat16, tag="tr")
                for g in range(g0, g0 + gn):
                    for kc in range(2):
                        ti = nc.tensor.transpose(
                            out=tp[:, (g - g0) * 2 + kc, :],
                            in_=eg1[:, g, kc * P:(kc + 1) * P],
                            identity=ident,
                        )
                        chain("pe", ti.ins)
                bank_copy(tp[:, 0:gn * 2, :], e1T[:, g0:g0 + gn, :, :])

            # per-unit staging tiles for the batched out DMAs
            stage = {}
            for u in UNIT_KEYS:
                stage[u] = constp.tile(
                    [P, max(Ks[u], 1), D_PROJ], mybir.dt.bfloat16,
                    tag=f"st{u}", name=f"st_{u}",
                )

            gbase_dev = {}
            acc_g = 0
            for u in UNIT_KEYS:
                gbase_dev[u] = acc_g
                acc_g += Ks[u]

            out_ct = [0]  # out-DMA engine round robin

            def emit_group(ch, lhsT_of, rhs_of, st, j):
                ps = psump.tile([P, 2 * NFREE], mybir.dt.float32, tag="ps")
                for kc in range(ch):
                    lt = lhsT_of(kc)
                    for oc in range(2):
                        mm = nc.tensor.matmul(
                            out=ps[:, oc * NFREE:(oc + 1) * NFREE],
                            lhsT=lt,
                            rhs=rhs_of(kc, oc),
                            start=(kc == 0),
                            stop=(kc == ch - 1),
                        )
                        chain("pe", mm.ins)
                bank_copy(ps[:], st[:, j, :])

            def emit_out(u, j0, k):
                """DMA groups [j0, j0+k) of unit u to HBM."""
                base = (gbase_dev[u] + j0) * P
                dst = out_ext[base:base + k * P, :].rearrange(
                    "(q p) d -> p q d", p=P
                )
                eng = nc.sync if out_ct[0] % 2 == 0 else nc.scalar
                inst = eng.dma_start(out=dst, in_=stage[u][:, j0:j0 + k, :])
                if out_ct[0] % 2 == 1:
                    chain("sca", inst.ins)
                out_ct[0] += 1

            def emit_unit_outs(u):
                K = Ks[u]
                for j0 in range(0, K, 2):
                    emit_out(u, j0, min(2, K - j0))

            # ---- c0 (8 contraction chunks) ----
            for j in range(K0):
                emit_group(
                    8,
                    lambda kc, _j=j: e0T[:, _j, kc, :],
                    lambda kc, oc: pt0_sb[:, kc, oc * NFREE:(oc + 1) * NFREE],
                    stage[0], j,
                )
            emit_unit_outs(0)
            # ---- c1 (2 chunks) ----
            for j in range(K1):
                emit_group(
                    2,
                    lambda kc, _j=j: e1T[:, _j, kc, :],
                    lambda kc, oc: pt1_sb[:, kc, oc * NFREE:(oc + 1) * NFREE],
                    stage[1], j,
                )
            emit_unit_outs(1)
            # ---- c2 (contraction 64; top 64 partitions of the padded rows
            # are unread) ----
            for r in range(C2_NSUB):
                for j in range(Ks[(2, r)]):
                    emit_group(
                        1,
                        lambda kc, _r=r, _j=j: e2s[_r][:64, _j, 0, :],
                        lambda kc, oc: pt2_sb[:64, oc * NFREE:(oc + 1) * NFREE],
                        stage[(2, r)], j,
                    )
                emit_unit_outs((2, r))
            # ---- c3: per-group mask-select of the 16-elem sub-row inside
            # the 128-elem packed super-row, then matmul vs the 8x-tiled
            # projection ----
            for j in range(K3):
                vec_op(
                    nc.vector.tensor_tensor,
                    out=em[:, j * P:(j + 1) * P],
                    in0=e3[:, j, 0, :],
                    in1=mask_v[:, j * P:(j + 1) * P],
                    op=mybir.AluOpType.mult,
                )
                emit_group(
                    1,
                    lambda kc, _j=j: em[:, _j * P:(_j + 1) * P],
                    lambda kc, oc: pt3_sb[:, oc * NFREE:(oc + 1) * NFREE],
                    stage[3], j,
                )
            emit_unit_outs(3)

    nc.compile()
    _GRAPH_CACHE[key] = nc
    return nc


def _wrap_idx16(vals, n_slots):
    """int16 values (len <= n_slots, padded with 0) -> [128, n_slots/16] wrapped."""
    full = np.zeros(n_slots, dtype=np.int16)
    full[:len(vals)] = vals
    w = np.zeros((16, n_slots // 16), dtype=np.int16)
    m = np.arange(n_slots)
    w[m % 16, m // 16] = full
    return np.tile(w, (8, 1))


def kernel(inp, emb0, emb1, emb2, emb3, proj0, proj1, proj2, proj3):
    inp = np.asarray(inp)
    embs = [np.asarray(e) for e in (emb0, emb1, emb2, emb3)]
    projs = [np.asarray(p) for p in (proj0, proj1, proj2, proj3)]
    B, S = inp.shape
    flat = inp.reshape(-1).astype(np.int64)
    T = flat.shape[0]

    # ---- host-side bucketing -------------------------------------------
    flat = np.clip(flat, 0, N_TOKEN - 1)
    cluster = np.clip(
        np.searchsorted(np.asarray(CUTOFF_ENDS[1:]), flat, side="right"), 0, 3
    )
    local = flat - np.asarray(CUTOFF_ENDS)[cluster]

    unit_pos = {}
    for u in UNIT_KEYS:
        if u == 0 or u == 1 or u == 3:
            unit_pos[u] = np.nonzero(cluster == u)[0]
        else:
            r = u[1]
            unit_pos[u] = np.nonzero((cluster == 2) & (local // C2_SUB == r))[0]

    core_lists = {u: [unit_pos[u][k::N_CORES] for k in range(N_CORES)]
                  for u in UNIT_KEYS}
    Ks = {
        u: int(-(-max(len(core_lists[u][k]) for k in range(N_CORES)) // P))
        for u in UNIT_KEYS
    }
    G = sum(Ks.values())
    K0, K1, K3 = Ks[0], Ks[1], Ks[3]

    NI32 = K0 + K1
    NI16 = 8 * sum(Ks[u] for u in UNITS_16)
    gbase = {}
    acc = 0
    for u in UNIT_KEYS:
        gbase[u] = acc
        acc += Ks[u]

    NCST = P + max(K3, 1) * P + D_PROJ + D_PROJ
    blkid = np.arange(128) // 16  # sub-row block of each super-row element

    # identity / pt2 / pt3s are core-independent
    pt2pad = np.zeros((P, D_PROJ), dtype=np.float32)
    pt2pad[:64] = projs[2].T.astype(np.float32) * EMB_SCALE
    pt3s = np.tile(projs[3].T.astype(np.float32) * EMB_SCALE, (C3_PACK, 1))

    idx32_maps, idx16_maps, cst_maps, row_maps = [], [], [], []
    for k in range(N_CORES):
        cols16 = []
        idx32 = np.zeros((P, max(NI32, 1)), dtype=np.int32)
        row_map = np.full(G * P, -1, dtype=np.int64)
        cst = np.zeros((P, NCST), dtype=np.float32)
        cst[:, 0:P] = np.eye(P, dtype=np.float32)
        cst[:, P + max(K3, 1) * P:P + max(K3, 1) * P + D_PROJ] = pt2pad
        cst[:, P + max(K3, 1) * P + D_PROJ:] = pt3s
        for u in UNIT_KEYS:
            n = Ks[u]
            if n == 0:
                continue
            lst = core_lists[u][k]
            m = np.arange(len(lst))
            row_map[(gbase[u] + m // P) * P + (m % P)] = lst
            if u == 0 or u == 1:
                base = 0 if u == 0 else K0
                full = np.zeros(n * P, dtype=np.int32)
                full[:len(lst)] = local[lst]
                idx32[:, base:base + n] = full.reshape(n, P).T
            else:
                lv = local[lst]
                if u == 3:
                    cols16.append(_wrap_idx16(
                        (lv // C3_PACK).astype(np.int16), n * P))
                    s_arr = lv % C3_PACK
                    mask = np.zeros((P, K3 * P), dtype=np.float32)
                    mask[:, m] = (blkid[:, None] == s_arr[None, :])
                    cst[:, P:P + K3 * P] = mask
                else:
                    cols16.append(_wrap_idx16(
                        (lv - u[1] * C2_SUB).astype(np.int16), n * P))
        idx16 = (np.concatenate(cols16, axis=1) if cols16
                 else np.zeros((P, 16), np.int16))
        if idx16.shape[1] < max(NI16, 16):
            pad = np.zeros((P, max(NI16, 16) - idx16.shape[1]), np.int16)
            idx16 = np.concatenate([idx16, pad], axis=1)
        idx32_maps.append(np.ascontiguousarray(idx32))
        idx16_maps.append(np.ascontiguousarray(idx16))
        cst_maps.append(cst.astype(BF16))
        row_maps.append(row_map)

    # ---- table/projection prep -----------------------------------------
    emb0b = np.ascontiguousarray(embs[0].astype(BF16))
    emb1b = np.ascontiguousarray(embs[1].astype(BF16))
    emb2p = np.zeros((C2_SUB * C2_NSUB, 128), dtype=BF16)
    emb2p[:160000, :64] = embs[2].astype(BF16)
    e3flat = embs[3].astype(np.float32)
    pad3 = C3_SROWS * C3_PACK - e3flat.shape[0]
    e3flat = np.concatenate([e3flat, np.zeros((pad3, 16), np.float32)], axis=0)
    emb3p = np.ascontiguousarray(e3flat.reshape(C3_SROWS, 128).astype(BF16))

    pts = {}
    for c, name, pc, ch in ((0, "pt0", 128, 8), (1, "pt1", 128, 2)):
        ptc = (projs[c].T.astype(np.float32) * EMB_SCALE).astype(BF16)
        pts[name] = np.ascontiguousarray(
            ptc.reshape(ch, pc, D_PROJ).transpose(1, 0, 2)
        )

    in_maps = []
    for k in range(N_CORES):
        m = {
            "idx32": idx32_maps[k], "idx16": idx16_maps[k], "cst": cst_maps[k],
            "emb0b": emb0b, "emb1b": emb1b, "emb2p": emb2p, "emb3p": emb3p,
        }
        m.update(pts)
        in_maps.append(m)

    # ---- device --------------------------------------------------------
    nc = _build_graph(Ks)
    res = run_bass_kernel_spmd(
        nc,
        in_maps,
        core_ids=list(range(N_CORES)),
        trace=TRACE,
        trace_cores=TRACE_CORES,
    )
    LAST["res"] = res
    LAST["Ks"] = Ks

    # ---- host-side unshard ---------------------------------------------
    out_full = np.zeros((T, D_PROJ), dtype=np.float32)
    for k in range(N_CORES):
        o = np.asarray(res.results[k]["out"])
        rm = row_maps[k]
        valid = rm >= 0
        out_full[rm[valid]] = o[valid].astype(np.float32)
    return out_full.reshape(B, S, D_PROJ)
